# revision 11
# baseline (speedup 1.0000x reference)
"""Trainium2 Bass kernel for nn_EnhancedQSelfAttention (B=8, C=512, H=W=64).

Strategy: data-parallel over batch — one batch element per NeuronCore (8
cores).  Per core, a flash-style two-pass quantized attention that never
materializes the 4096x4096 attention matrix in HBM:

  pass 1:  attn tiles [i,m] = (0.125*q)^T k  (bf16 matmul), row max/min
           tracked with fused tensor_tensor_reduce ops.
  global:  AllReduce(min) of min_i(rowmin_i - rowmax_i) over the 8 cores
           -> emin = exp(d); emax = 1 exactly (each row contains exp(0)).
  pass 2:  attn'^T tiles [m,i] via an augmented K=65 matmul whose extra
           contraction row carries (ln(scale) - m_i), so ScalarE's Exp
           directly produces scale*e in fp16; quantization via the fp16
           magic-number rounding (+ (1024 - zp), -1024, min 255) on
           VectorE; quantized integers (exact in bf16) feed the PV
           matmuls in [c,i] layout plus a ones-row matmul for rowsums.
  epilog:  affine zero-point correction + normalization, output
           projection (gamma folded into wo on host), residual add.
"""

import numpy as np
import ml_dtypes

import concourse.bass as bass
import concourse.tile as tile
from concourse import mybir
from concourse.bass_utils import run_bass_kernel_spmd

F32 = mybir.dt.float32
BF16 = mybir.dt.bfloat16
F16 = mybir.dt.float16
AOP = mybir.AluOpType
ACT = mybir.ActivationFunctionType

B, C, H, W = 8, 512, 64, 64
N = H * W            # 4096
CK = 64
QMAX = 255.0
ATTN_SCALE = CK ** -0.5   # 0.125
NCORES = 8

nbf = ml_dtypes.bfloat16


# ---------------------------------------------------------------- IR fixup
def _split_waits(nc, maxw=1):
    """This walrus build rejects >1 sem-wait per CTRL instruction
    ("Too many sync wait commands").  Hoist excess waits onto same-engine
    nops inserted immediately before the offending instruction."""
    for fn in nc.m.functions:
        for bb in fn.blocks:
            insts = list(bb.instructions)
            if not any(
                i.sync_info and i.sync_info.on_wait and len(i.sync_info.on_wait) > maxw
                for i in insts
            ):
                continue
            newlist = []
            appended = set()
            for inst in insts:
                si = inst.sync_info
                if si and si.on_wait and len(si.on_wait) > maxw:
                    waits = list(si.on_wait)
                    excess, keep = waits[:-maxw], waits[-maxw:]
                    eng = nc.engines[inst.engine]
                    for j in range(0, len(excess), maxw):
                        grp = excess[j : j + maxw]
                        ni = eng.nop(nofuse=True, hint="wait_split").ins
                        ni.sync_info = mybir.SyncInfo(on_wait=grp, on_update=[])
                        appended.add(ni.name)
                        newlist.append(ni)
                    inst.sync_info = mybir.SyncInfo(
                        on_wait=keep, on_update=list(si.on_update or [])
                    )
                newlist.append(inst)
            bb.instructions = newlist
            if appended:
                # eng.nop auto-appended the new nops to nc.cur_bb; drop those
                # stray copies everywhere except the position we placed them.
                for fb in fn.blocks:
                    lst = list(fb.instructions)
                    seen = set()
                    cleaned = []
                    for x in lst:
                        if x.name in appended:
                            if fb.name != bb.name or x.name in seen:
                                continue
                            seen.add(x.name)
                        cleaned.append(x)
                    if len(cleaned) != len(lst):
                        fb.instructions = cleaned


# ---------------------------------------------------------------- builder
def _build_nc():
    nc = bass.Bass("TRN2", target_bir_lowering=False, debug=False,
                   num_devices=NCORES)

    # ---- kernel I/O (per core) ----
    x_d = nc.dram_tensor("x", [C, N], F32, kind="ExternalInput").ap()
    wqT_d = nc.dram_tensor("wqT", [C, CK], BF16, kind="ExternalInput").ap()
    wkT_d = nc.dram_tensor("wkT", [C, CK], BF16, kind="ExternalInput").ap()
    wvT_d = nc.dram_tensor("wvT", [C, C], BF16, kind="ExternalInput").ap()
    woT_d = nc.dram_tensor("woTg", [C, C], BF16, kind="ExternalInput").ap()
    bq_d = nc.dram_tensor("bq_s", [CK, 1], F32, kind="ExternalInput").ap()
    bk_d = nc.dram_tensor("bk_c", [CK, 1], F32, kind="ExternalInput").ap()
    bv_d = nc.dram_tensor("bv_r", [1, C], BF16, kind="ExternalInput").ap()
    bog_d = nc.dram_tensor("bog_c", [128, 4], F32, kind="ExternalInput").ap()
    out_d = nc.dram_tensor("out", [C, N], F32, kind="ExternalOutput").ap()

    with tile.TileContext(nc) as tc:
        with (
            tc.tile_pool(name="persist", bufs=1) as pp,
            tc.tile_pool(name="dram", bufs=1, space="DRAM") as dp,
        ):
            # ---- persistent SBUF tiles ----
            x_bf = pp.tile([128, 4 * N], BF16)       # x (ch-blk kt major)
            q_sb = pp.tile([128, N], BF16)           # rows 0..63 q', row 64 aug
            k_sb = pp.tile([128, N], BF16)           # rows 0..63 k, row 64 ones
            vT_sb = pp.tile([128, 32 * C], BF16)     # v^T  (m-blk major)
            att_sb = pp.tile([128, 4 * N], BF16)     # att [c,i] (cb major)
            wqT_sb = pp.tile([128, 4 * CK], BF16)
            wkT_sb = pp.tile([128, 4 * CK], BF16)
            wvT_sb = pp.tile([128, 4 * C], BF16)
            woT_sb = pp.tile([128, 4 * C], BF16)
            bq_sb = pp.tile([CK, 1], F32)
            bk_sb = pp.tile([CK, 1], F32)
            bv_bc = pp.tile([128, C], BF16)
            bog_sb = pp.tile([128, 4], F32)
            ones_col = pp.tile([128, 1], BF16)
            mcol = pp.tile([128, 32], F32)           # row max
            rcol = pp.tile([128, 32], F32)           # row min
            vsum_row = pp.tile([1, C], F32)
            vsum_col = pp.tile([128, 4], F32)
            zpv = pp.tile([128, 4], F32)
            cc_in_sb = pp.tile([1, 8], F32)
            # scalar chain tiles
            dglob = pp.tile([1, 1], F32)
            emin_t = pp.tile([1, 1], F32)
            om_t = pp.tile([1, 1], F32)
            rec_t = pp.tile([1, 1], F32)
            scale_t = pp.tile([1, 1], F32)
            cln_t = pp.tile([1, 1], F32)
            zp_t = pp.tile([1, 1], F32)
            zpk_t = pp.tile([1, 1], F32)
            nzp_t = pp.tile([1, 1], F32)
            zpk_bc = pp.tile([128, 1], F32)
            cln_bc = pp.tile([128, 1], F32)
            zp_bc = pp.tile([128, 1], F32)
            aug_col = pp.tile([128, 32], BF16)
            dif_t = pp.tile([128, 32], F32)
            dvec_t = pp.tile([128, 1], F32)
            drow_t = pp.tile([1, 128], F32)
            dred_t = pp.tile([1, 1], F32)

            # ---- DRAM scratch ----
            aug_dram = dp.tile([N], BF16)
            d128_dram = dp.tile([128], F32)
            vs_dram = dp.tile([C], F32)
            cc_in_dram = dp.tile([1, 8], F32)
            cc_out_dram = dp.tile([1, 8], F32, addr_space="Shared")
            zpk_dram = dp.tile([1], F32)
            cln_dram = dp.tile([1], F32)
            zpsc_dram = dp.tile([1], F32)
            rd_dram = dp.tile([C], F32)

            # ================= P0: weight loads + constants =================
            for kt in range(4):
                nc.sync.dma_start(wqT_sb[:, kt * CK:(kt + 1) * CK],
                                  wqT_d[kt * 128:(kt + 1) * 128, :])
                nc.sync.dma_start(wkT_sb[:, kt * CK:(kt + 1) * CK],
                                  wkT_d[kt * 128:(kt + 1) * 128, :])
                nc.sync.dma_start(wvT_sb[:, kt * C:(kt + 1) * C],
                                  wvT_d[kt * 128:(kt + 1) * 128, :])
                nc.sync.dma_start(woT_sb[:, kt * C:(kt + 1) * C],
                                  woT_d[kt * 128:(kt + 1) * 128, :])
            nc.sync.dma_start(bq_sb[:], bq_d[:])
            nc.sync.dma_start(bk_sb[:], bk_d[:])
            nc.sync.dma_start(bog_sb[:], bog_d[:])
            nc.sync.dma_start(bv_bc[:], bv_d[:].to_broadcast((128, C)))
            nc.vector.memset(ones_col[:], 1.0)
            nc.vector.memset(k_sb[64:65, :], 1.0)
            nc.vector.memset(mcol[:], -1e30)
            nc.vector.memset(rcol[:], 1e30)
            nc.vector.memset(cc_in_sb[:], 1e30)

            # x load + cast to bf16
            with tc.tile_pool(name="xload", bufs=2) as xp:
                for kt in range(4):
                    xf = xp.tile([128, N], F32)
                    nc.sync.dma_start(xf[:], x_d[kt * 128:(kt + 1) * 128, :])
                    nc.vector.tensor_copy(x_bf[:, kt * N:(kt + 1) * N], xf[:])

            # ================= P1: projections =================
            with (
                tc.tile_pool(name="pqk", bufs=2, space="PSUM") as pqk_pool,
                tc.tile_pool(name="pv", bufs=2, space="PSUM") as pv_pool,
                tc.tile_pool(name="pvs", bufs=1, space="PSUM") as pvs_pool,
            ):
                for nt in range(8):
                    s = slice(nt * 512, (nt + 1) * 512)
                    pq = pqk_pool.tile([CK, 512], F32, tag="pq")
                    for kt in range(4):
                        nc.tensor.matmul(
                            pq[:], wqT_sb[:, kt * CK:(kt + 1) * CK],
                            x_bf[:, kt * N + nt * 512: kt * N + (nt + 1) * 512],
                            start=(kt == 0), stop=(kt == 3))
                    nc.vector.tensor_scalar(
                        out=q_sb[0:CK, s], in0=pq[:], scalar1=bq_sb[:],
                        scalar2=None, op0=AOP.add)
                    pk = pqk_pool.tile([CK, 512], F32, tag="pq")
                    for kt in range(4):
                        nc.tensor.matmul(
                            pk[:], wkT_sb[:, kt * CK:(kt + 1) * CK],
                            x_bf[:, kt * N + nt * 512: kt * N + (nt + 1) * 512],
                            start=(kt == 0), stop=(kt == 3))
                    nc.vector.tensor_scalar(
                        out=k_sb[0:CK, s], in0=pk[:], scalar1=bk_sb[:],
                        scalar2=None, op0=AOP.add)

                # v^T blocks
                for mb in range(32):
                    pv = pv_pool.tile([128, C], F32)
                    for kt in range(4):
                        nc.tensor.matmul(
                            pv[:],
                            x_bf[:, kt * N + mb * 128: kt * N + (mb + 1) * 128],
                            wvT_sb[:, kt * C:(kt + 1) * C],
                            start=(kt == 0), stop=(kt == 3))
                    nc.vector.scalar_tensor_tensor(
                        out=vT_sb[:, mb * C:(mb + 1) * C], in0=pv[:],
                        scalar=0.0, in1=bv_bc[:], op0=AOP.bypass, op1=AOP.add)

                # Vsum_c = sum_m v[m,c] via ones-matmul
                pvs = pvs_pool.tile([1, C], F32)
                for mb in range(32):
                    nc.tensor.matmul(pvs[:], ones_col[:],
                                     vT_sb[:, mb * C:(mb + 1) * C],
                                     start=(mb == 0), stop=(mb == 31))
                nc.vector.tensor_copy(vsum_row[:], pvs[:])
            nc.sync.dma_start(vs_dram[:].rearrange("(a p) -> a p", a=1),
                              vsum_row[:])
            nc.sync.dma_start(vsum_col[:], vs_dram[:].rearrange("(a p) -> p a", p=128))

            # ================= P2: pass-1 stats =================
            with (
                tc.tile_pool(name="pa", bufs=3, space="PSUM") as pa_pool,
                tc.tile_pool(name="abf", bufs=3) as abf_pool,
                tc.tile_pool(name="acc", bufs=2) as acc_pool,
            ):
                for ib in range(32):
                    macc = acc_pool.tile([128, 512], BF16, tag="macc")
                    racc = acc_pool.tile([128, 512], BF16, tag="racc")
                    for mt in range(8):
                        pa = pa_pool.tile([128, 512], F32)
                        nc.tensor.matmul(
                            pa[:], q_sb[0:CK, ib * 128:(ib + 1) * 128],
                            k_sb[0:CK, mt * 512:(mt + 1) * 512],
                            start=True, stop=True)
                        abf = abf_pool.tile([128, 512], BF16)
                        nc.scalar.activation(abf[:], pa[:], ACT.Copy)
                        if mt == 0:
                            nc.vector.tensor_copy(macc[:], abf[:])
                            nc.vector.tensor_copy(racc[:], abf[:])
                        else:
                            nc.vector.tensor_tensor(macc[:], macc[:], abf[:],
                                                    op=AOP.max)
                            nc.vector.tensor_tensor(racc[:], racc[:], abf[:],
                                                    op=AOP.min)
                    nc.vector.tensor_reduce(mcol[:, ib:ib + 1], macc[:],
                                            axis=mybir.AxisListType.X,
                                            op=AOP.max)
                    nc.vector.tensor_reduce(rcol[:, ib:ib + 1], racc[:],
                                            axis=mybir.AxisListType.X,
                                            op=AOP.min)

            # d_core = min(rcol - mcol) -> scalar
            nc.vector.tensor_sub(dif_t[:], rcol[:], mcol[:])
            nc.vector.tensor_reduce(dvec_t[:], dif_t[:],
                                    axis=mybir.AxisListType.X, op=AOP.min)
            nc.sync.dma_start(d128_dram[:].rearrange("(p a) -> p a", a=1),
                              dvec_t[:])
            nc.sync.dma_start(drow_t[:],
                              d128_dram[:].rearrange("(a p) -> a p", a=1))
            nc.vector.tensor_reduce(dred_t[:], drow_t[:],
                                    axis=mybir.AxisListType.X, op=AOP.min)
            nc.vector.tensor_copy(cc_in_sb[0:1, 0:1], dred_t[:])

            # ---- AllReduce(min) over 8 cores ----
            nc.sync.dma_start(cc_in_dram[:], cc_in_sb[:])
            nc.gpsimd.collective_compute(
                "AllReduce", AOP.min,
                ins=[cc_in_dram.opt()], outs=[cc_out_dram.opt()],
                replica_groups=[list(range(NCORES))])
            nc.sync.dma_start(dglob[:], cc_out_dram[0:1, 0:1])

            # ---- scalar chain ----
            nc.scalar.activation(emin_t[:], dglob[:], ACT.Exp)
            nc.vector.tensor_scalar(out=om_t[:], in0=emin_t[:], scalar1=-1.0,
                                    scalar2=1.0, op0=AOP.mult, op1=AOP.add)
            nc.vector.reciprocal(rec_t[:], om_t[:])
            nc.vector.tensor_scalar(out=scale_t[:], in0=rec_t[:],
                                    scalar1=QMAX, scalar2=None, op0=AOP.mult)
            nc.scalar.activation(cln_t[:], scale_t[:], ACT.Ln)
            nc.vector.tensor_tensor(zp_t[:], scale_t[:], emin_t[:],
                                    op=AOP.mult)
            nc.vector.tensor_scalar(out=zpk_t[:], in0=zp_t[:], scalar1=-1.0,
                                    scalar2=1024.0, op0=AOP.mult, op1=AOP.add)
            nc.vector.tensor_scalar(out=nzp_t[:], in0=zp_t[:], scalar1=4096.0,
                                    scalar2=None, op0=AOP.mult)
            nc.sync.dma_start(zpk_dram[:].rearrange("(a p) -> a p", a=1),
                              zpk_t[:])
            nc.sync.dma_start(zpk_bc[:], zpk_dram[:].to_broadcast((128, 1)))
            nc.sync.dma_start(cln_dram[:].rearrange("(a p) -> a p", a=1),
                              cln_t[:])
            nc.sync.dma_start(cln_bc[:], cln_dram[:].to_broadcast((128, 1)))
            nc.sync.dma_start(zpsc_dram[:].rearrange("(a p) -> a p", a=1),
                              zp_t[:])
            nc.sync.dma_start(zp_bc[:], zpsc_dram[:].to_broadcast((128, 1)))
            nc.vector.tensor_scalar(out=zpv[:], in0=vsum_col[:],
                                    scalar1=zp_bc[:], scalar2=None,
                                    op0=AOP.mult)

            # aug row: q_sb[64, i] = ln(scale) - m_i
            nc.vector.tensor_scalar(out=aug_col[:], in0=mcol[:], scalar1=-1.0,
                                    scalar2=cln_bc[:], op0=AOP.mult,
                                    op1=AOP.add)
            nc.sync.dma_start(aug_dram[:].rearrange("(a p) -> p a", p=128),
                              aug_col[:])
            nc.sync.dma_start(q_sb[64:65, :],
                              aug_dram[:].rearrange("(a p) -> a p", a=1))

            # ================= P3: pass-2 =================
            with (
                tc.tile_pool(name="pqk2", bufs=2, space="PSUM") as pqk2_pool,
                tc.tile_pool(name="patt", bufs=1, space="PSUM") as patt_pool,
                tc.tile_pool(name="pseq", bufs=1, space="PSUM") as pseq_pool,
                tc.tile_pool(name="e16", bufs=3) as e_pool,
                tc.tile_pool(name="u16", bufs=3) as u_pool,
                tc.tile_pool(name="eqb", bufs=3) as eq_pool,
                tc.tile_pool(name="norm", bufs=2) as n_pool,
            ):
                for it in range(8):
                    isl = slice(it * 512, (it + 1) * 512)
                    att_ps = [patt_pool.tile([128, 512], F32, tag=f"att{cb}",
                                             name=f"att_ps{cb}")
                              for cb in range(4)]
                    seq_ps = pseq_pool.tile([1, 512], F32)
                    for mb in range(32):
                        pqk = pqk2_pool.tile([128, 512], F32)
                        nc.tensor.matmul(
                            pqk[:], k_sb[0:CK + 1, mb * 128:(mb + 1) * 128],
                            q_sb[0:CK + 1, isl], start=True, stop=True)
                        e16 = e_pool.tile([128, 512], F16)
                        nc.scalar.activation(e16[:], pqk[:], ACT.Exp)
                        u16 = u_pool.tile([128, 512], F16)
                        nc.vector.tensor_scalar(
                            out=u16[:], in0=e16[:], scalar1=zpk_bc[:],
                            scalar2=None, op0=AOP.add)
                        eqb = eq_pool.tile([128, 512], BF16)
                        nc.vector.tensor_scalar(
                            out=eqb[:], in0=u16[:], scalar1=1024.0,
                            scalar2=QMAX, op0=AOP.subtract, op1=AOP.min)
                        for cb in range(4):
                            nc.tensor.matmul(
                                att_ps[cb][:],
                                vT_sb[:, mb * C + cb * 128: mb * C + (cb + 1) * 128],
                                eqb[:], start=(mb == 0), stop=(mb == 31))
                        nc.tensor.matmul(seq_ps[:], ones_col[:], eqb[:],
                                         start=(mb == 0), stop=(mb == 31))
                    den = n_pool.tile([1, 512], F32, tag="den")
                    nc.vector.tensor_scalar(out=den[:], in0=seq_ps[:],
                                            scalar1=nzp_t[:], scalar2=None,
                                            op0=AOP.add)
                    rden = n_pool.tile([1, 512], F32, tag="rden")
                    nc.vector.reciprocal(rden[:], den[:])
                    nc.sync.dma_start(
                        rd_dram[:].rearrange("(a p) -> a p", a=1), rden[:])
                    rden128 = n_pool.tile([128, 512], F32, tag="rden128")
                    nc.sync.dma_start(
                        rden128[:],
                        rd_dram[:].rearrange("(a p) -> a p", a=1)
                        .to_broadcast((128, C)))
                    for cb in range(4):
                        nc.vector.scalar_tensor_tensor(
                            out=att_sb[:, cb * N + it * 512: cb * N + (it + 1) * 512],
                            in0=att_ps[cb][:], scalar=zpv[:, cb:cb + 1],
                            op0=AOP.add, in1=rden128[:], op1=AOP.mult)

            # ================= P4: output projection + residual =================
            with (
                tc.tile_pool(name="pf", bufs=2, space="PSUM") as pf_pool,
                tc.tile_pool(name="xres", bufs=3) as xr_pool,
                tc.tile_pool(name="osb", bufs=3) as o_pool,
            ):
                for ob in range(4):
                    for nt in range(8):
                        pf = pf_pool.tile([128, 512], F32)
                        for cb in range(4):
                            nc.tensor.matmul(
                                pf[:],
                                woT_sb[:, cb * C + ob * 128: cb * C + (ob + 1) * 128],
                                att_sb[:, cb * N + nt * 512: cb * N + (nt + 1) * 512],
                                start=(cb == 0), stop=(cb == 3))
                        xres = xr_pool.tile([128, 512], F32)
                        nc.sync.dma_start(
                            xres[:], x_d[ob * 128:(ob + 1) * 128,
                                         nt * 512:(nt + 1) * 512])
                        outt = o_pool.tile([128, 512], F32)
                        nc.vector.scalar_tensor_tensor(
                            out=outt[:], in0=pf[:],
                            scalar=bog_sb[:, ob:ob + 1], op0=AOP.add,
                            in1=xres[:], op1=AOP.add)
                        nc.sync.dma_start(
                            out_d[ob * 128:(ob + 1) * 128,
                                  nt * 512:(nt + 1) * 512], outt[:])

    _split_waits(nc)
    return nc


_NC_CACHE = {}


def _get_nc():
    if "nc" not in _NC_CACHE:
        _NC_CACHE["nc"] = _build_nc()
    return _NC_CACHE["nc"]


def kernel(**inputs):
    x = np.asarray(inputs["x"], np.float32)          # [8, 512, 64, 64]
    wq = np.asarray(inputs["wq"], np.float32)
    bq = np.asarray(inputs["bq"], np.float32)
    wk = np.asarray(inputs["wk"], np.float32)
    bk = np.asarray(inputs["bk"], np.float32)
    wv = np.asarray(inputs["wv"], np.float32)
    bv = np.asarray(inputs["bv"], np.float32)
    wo = np.asarray(inputs["wo"], np.float32)
    bo = np.asarray(inputs["bo"], np.float32)
    gamma = float(np.asarray(inputs["gamma"]).reshape(-1)[0])

    wqT = np.ascontiguousarray((wq * ATTN_SCALE).T).astype(nbf)   # [512, 64]
    wkT = np.ascontiguousarray(wk.T).astype(nbf)                  # [512, 64]
    wvT = np.ascontiguousarray(wv.T).astype(nbf)                  # [512, 512]
    woTg = np.ascontiguousarray((gamma * wo).T).astype(nbf)       # [512, 512]
    bq_s = (bq * ATTN_SCALE).reshape(CK, 1).astype(np.float32)
    bk_c = bk.reshape(CK, 1).astype(np.float32)
    bv_r = bv.reshape(1, C).astype(nbf)
    bog_c = np.ascontiguousarray((gamma * bo).reshape(4, 128).T).astype(np.float32)

    nc = _get_nc()
    in_maps = []
    for b in range(B):
        in_maps.append({
            "x": np.ascontiguousarray(x[b].reshape(C, N)),
            "wqT": wqT, "wkT": wkT, "wvT": wvT, "woTg": woTg,
            "bq_s": bq_s, "bk_c": bk_c, "bv_r": bv_r, "bog_c": bog_c,
        })
    res = run_bass_kernel_spmd(nc, in_maps, list(range(NCORES)))
    out = np.stack([np.asarray(res.results[b]["out"], np.float32)
                    .reshape(C, H, W) for b in range(B)])
    return out
